# revision 7
# baseline (speedup 1.0000x reference)
"""AGNN message-passing kernel for 8 TRN2 NeuronCores (Bass/Tile).

Sharding: nodes dst-sharded 8 ways; edges colocated with their target node
(sorted per dst-group / src-chunk / dst).  Per conv layer: source features
gathered Q7-group-parallel with ap_gather from fp32 [hn|h] feature-major
chunk tables; dst features expanded by a boundary-reset DVE prefix scan fed
by GPSIMD local_scatter (no per-edge dst gather); self-loop contributions
added in closed form (exp(beta*||hn||^2)*[h|1]) so they need no edge slots;
attention = replicated-alpha matmuls + ScalarE exp; scatter-softmax segment
sums via DVE cumsum + ap_gather boundary diffs; tables exchanged with
AllGather; global max/avg pool via masked max-scan / cumsum + AllGather.
Host side does index/layout preprocessing only.
"""
import time
import numpy as np
import jax
from jax.sharding import Mesh, PartitionSpec
from jax.experimental.shard_map import shard_map
from concourse import bass, mybir, bacc
from concourse import bass2jax
from concourse.bass2jax import _bass_exec_p, install_neuronx_cc_hook, partition_id_tensor
import concourse.tile as tile

F16 = mybir.dt.float16
F32 = mybir.dt.float32
I16 = mybir.dt.int16
ALU = mybir.AluOpType
ACT = mybir.ActivationFunctionType


def r16(x):
    return ((x + 15) // 16) * 16


class Cfg:
    def __init__(self, N=100000, E=3200000, D=128, G=64, NC=8):
        self.N, self.E, self.D, self.G, self.NC = N, E, D, G, NC
        self.NPC = N // NC            # nodes per core
        self.NPG = (self.NPC + 7) // 8  # nodes per Q7 group
        self.ENDW = r16(self.NPG + 1)   # ends list width per chunk
        self.SUMW = r16(G + 1)          # sum-pool idx width
        self.MAXW = r16(G)              # max-pool idx width
        self.B = 512                    # edge block size


def preprocess(cfg, x, edge_index, batch, W1, b1, beta2, W2, b2, W3, b3):
    """Pure index/layout preprocessing. Returns (in_maps list per core, meta dict)."""
    N, NC, NPC, NPG = cfg.N, cfg.NC, cfg.NPC, cfg.NPG
    src = np.asarray(edge_index[0], dtype=np.int64)
    dst = np.asarray(edge_index[1], dtype=np.int64)
    batch = np.asarray(batch, dtype=np.int64)

    core = dst // NPC
    dstl = dst - core * NPC
    grp = dstl // NPG
    chunk = src // NPC
    order = np.lexsort((dst, chunk, grp, core))
    src, dst, core, dstl, grp, chunk = (a[order] for a in (src, dst, core, dstl, grp, chunk))
    srcl = src - chunk * NPC

    # counts per (core, grp, chunk)
    key = (core * 8 + grp) * NC + chunk
    cnt = np.bincount(key, minlength=NC * 8 * NC).reshape(NC, 8, NC)
    Rc = np.array([((cnt[:, :, c].max() + 4 + 63) // 64) * 64 for c in range(NC)], dtype=np.int64)
    Roff = np.concatenate([[0], np.cumsum(Rc)])
    L = int(Roff[-1])          # per-group edge array length (uniform)
    SI = L // 16

    starts = np.zeros(NC * 8 * NC, dtype=np.int64)
    np.cumsum(cnt.reshape(-1)[:-1], out=starts[1:])
    # position within (core,grp,chunk) block
    within = np.arange(len(src)) - starts[key]
    pos = Roff[chunk] + 4 + within  # position in the group's edge array

    part = (grp * 16 + pos % 16).astype(np.int64)
    col = (pos // 16).astype(np.int64)

    srcidx = np.zeros((NC, 128, SI), dtype=np.int16)
    srcidx[core, part, col] = srcl.astype(np.int16)

    # --- dst-side scan-expand metadata ---
    B = cfg.B
    dmask = np.ones((NC, 128, L), dtype=np.float16)
    # run starts: first edge of each (core,grp,chunk,node) run
    gl = dstl - grp * NPG  # group-local node
    first = np.ones(len(src), dtype=bool)
    first[1:] = (key[1:] != key[:-1]) | (gl[1:] != gl[:-1])
    sp = pos[first]
    dmask[core[first][:, None], (grp[first] * 16)[:, None] + np.arange(16)[None, :], sp[:, None]] = 0.0
    # per (chunk, block): node span [j0, j0+K) uniform across cores/groups; scatter idx lists
    blocks = []
    for c in range(NC):
        for a in range(0, int(Rc[c]), B):
            blocks.append((c, a, min(B, int(Rc[c]) - a)))
    # node range per (core,grp,chunk,block): nodes of runs starting in block, plus carry-in node
    scat_parts = []
    KBLK = []
    # precompute per-edge block id within chunk
    blkid = within + 4  # position within chunk region
    for (c, a, W) in blocks:
        sel = (chunk == c) & (blkid >= a) & (blkid < a + W)
        j0 = 0; j1 = 1
        if sel.any():
            j0 = int(gl[sel].min()); j1 = int(gl[sel].max()) + 1
        K = max(j1 - j0, 1)
        KBLK.append((c, a, W, j0, K))
    KMAX = max(k[4] for k in KBLK)
    KMAX = ((KMAX + 15) // 16) * 16
    SCATW = KMAX * len(KBLK)
    scatidx = np.full((NC, 128, SCATW), -1, dtype=np.int16)
    sc_chunk = chunk[first]; sc_core = core[first]; sc_grp = grp[first]; sc_gl = gl[first]
    sc_blk = blkid[first] // B  # block index within chunk
    nblk_per_chunk = [int((Rc[c] + B - 1) // B) for c in range(NC)]
    cum_blk = np.concatenate([[0], np.cumsum(nblk_per_chunk)])
    j0_arr = np.zeros(len(KBLK), np.int64)
    for bi, (c, a, W, j0, K) in enumerate(KBLK):
        j0_arr[bi] = j0
    gb = cum_blk[sc_chunk] + sc_blk  # global block id per run-start
    rel = sc_gl - j0_arr[gb]
    okm = (rel >= 0) & (rel < KMAX)
    scatidx[sc_core[okm][:, None], (sc_grp[okm] * 16)[:, None] + np.arange(16)[None, :],
            (gb[okm] * KMAX + rel[okm])[:, None]] = ((blkid[first][okm] - (sc_blk[okm] * B))[:, None]).astype(np.int16)
    meta_blocks = [(c, a, W, int(j0_arr[bi]), KMAX) for bi, (c, a, W, j0, K) in enumerate(KBLK)]

    # ends per (core, grp, chunk): [3, 3+cum(0), ..., 3+cum(NPG-1)] padded to ENDW
    # cum over nodes of the group within this chunk
    nodecnt = np.bincount((core * 8 + grp) * (NC * NPG) + chunk * NPG + (dstl - grp * NPG),
                          minlength=NC * 8 * NC * NPG).reshape(NC, 8, NC, NPG)
    cum = np.cumsum(nodecnt, axis=3)
    ends = np.zeros((NC, 8, NC, cfg.ENDW), dtype=np.int64)
    ends[:, :, :, 0] = 3
    ends[:, :, :, 1:NPG + 1] = 3 + cum
    ends[:, :, :, NPG + 1:] = ends[:, :, :, NPG:NPG + 1]
    endsidx = np.zeros((NC, 128, NC * (cfg.ENDW // 16)), dtype=np.int16)
    for g in range(8):
        for c in range(NC):
            e = ends[:, g, c, :]  # [NC, ENDW]
            w = e.reshape(NC, cfg.ENDW // 16, 16).transpose(0, 2, 1)
            endsidx[:, 16 * g:16 * g + 16, c * (cfg.ENDW // 16):(c + 1) * (cfg.ENDW // 16)] = w

    # pooling: node -> graph, per (core, grp): counts per graph
    gnode = batch  # [N]
    nodecore = np.arange(N) // NPC
    nodegrp = (np.arange(N) % NPC) // NPG
    pk = (nodecore * 8 + nodegrp) * cfg.G + gnode
    pcnt = np.bincount(pk, minlength=NC * 8 * cfg.G).reshape(NC, 8, cfg.G)
    pcum = np.cumsum(pcnt, axis=2)
    sumends = np.zeros((NC, 8, cfg.SUMW), dtype=np.int64)
    sumends[:, :, 0] = 3
    sumends[:, :, 1:cfg.G + 1] = 3 + pcum
    sumends[:, :, cfg.G + 1:] = sumends[:, :, cfg.G:cfg.G + 1]
    maxends = np.where(pcnt > 0, 3 + pcum, 3)  # absent -> sentinel 3 (value 0)
    sumidx = np.zeros((NC, 128, cfg.SUMW // 16), dtype=np.int16)
    maxidx = np.zeros((NC, 128, cfg.MAXW // 16), dtype=np.int16)
    for g in range(8):
        sumidx[:, 16 * g:16 * g + 16, :] = sumends[:, g].reshape(NC, cfg.SUMW // 16, 16).transpose(0, 2, 1)
        me = np.zeros((NC, cfg.MAXW), dtype=np.int64)
        me[:, :cfg.G] = maxends[:, g]
        maxidx[:, 16 * g:16 * g + 16, :] = me.reshape(NC, cfg.MAXW // 16, 16).transpose(0, 2, 1)

    # graph-start mask per (core, grp): zeros at first col of each present graph, and col 0
    gmask = np.ones((NC, 128, NPG), dtype=np.float16)
    gstart = np.zeros((NC, 8, NPG), dtype=bool)
    gstart[:, :, 0] = True
    prev = np.concatenate([np.zeros((NC, 8, 1), np.int64), pcum[:, :, :-1]], axis=2)
    for r in range(NC):
        for g in range(8):
            s = prev[r, g][pcnt[r, g] > 0]
            s = s[s < NPG]
            gstart[r, g, s] = True
    for g in range(8):
        gmask[:, 16 * g:16 * g + 16, :] = np.where(gstart[:, g], 0.0, 1.0)[:, None, :]

    gcnt = np.maximum(np.bincount(batch, minlength=cfg.G).astype(np.float32), 1.0)
    RC = np.ones((32, cfg.G), dtype=np.float32)
    RC[16:32, :] = (1.0 / gcnt)[None, :]

    # constant matrices
    LA = np.zeros((128, 128), np.float16)
    LB1 = np.zeros((128, 128), np.float16)
    LB2 = np.zeros((128, 128), np.float16)
    PERM16 = np.zeros((128, 128), np.float16)
    SUMPERM = np.zeros((128, 128), np.float16)
    for g in range(8):
        b = 16 * g
        LA[b:b + 8, b:b + 16] = 1.0
        for r in range(8):
            LB1[b + 8 + r, b + r] = 1.0
            LB2[b + 8 + r, b + 8 + r] = 1.0
        PERM16[b + 8, b:b + 16] = 1.0
        SUMPERM[b:b + 8, b:b + 16] = 1.0
    W2BD = np.zeros((128, 128), np.float16)
    b2rep = np.zeros((128, 1), np.float32)
    for g in range(8):
        W2BD[16 * g:16 * g + 8, 16 * g:16 * g + 16] = np.asarray(W1, np.float16)[:8, :16] if False else np.asarray(W2, np.float16)
        b2rep[16 * g:16 * g + 16, 0] = np.asarray(b2, np.float32)
    betavec = np.full((128, 1), float(np.asarray(beta2)), np.float32)
    hnmask_np = np.zeros((128, 1), np.float32)
    onemask_np = np.ones((128, 1), np.float32)
    for g in range(8):
        hnmask_np[16 * g:16 * g + 8, 0] = 1.0
        onemask_np[16 * g:16 * g + 8, 0] = 0.0

    # ---- pack everything into 3 input tensors (NEFF launch pays ~45us per
    # input tensor, so input count dominates the fixed per-run overhead) ----
    SCATW = scatidx.shape[2]
    WI = r16(SI + SCATW + NC * (cfg.ENDW // 16) + cfg.SUMW // 16 + cfg.MAXW // 16)
    WC = r16(NPC + L + cfg.NPG + 6 * 128 + 8 + 2)
    WS = r16(6 + cfg.G)

    xf = np.asarray(x, np.float32)
    in_maps = []
    for r in range(NC):
        xT16 = np.ascontiguousarray(xf[r * NPC:(r + 1) * NPC, :].T).astype(np.float16)
        IDXP = np.zeros((128, WI), np.int16)
        o = 0
        IDXP[:, o:o + SI] = srcidx[r]; o += SI
        IDXP[:, o:o + SCATW] = scatidx[r]; o += SCATW
        IDXP[:, o:o + NC * (cfg.ENDW // 16)] = endsidx[r]; o += NC * (cfg.ENDW // 16)
        IDXP[:, o:o + cfg.SUMW // 16] = sumidx[r]; o += cfg.SUMW // 16
        IDXP[:, o:o + cfg.MAXW // 16] = maxidx[r]; o += cfg.MAXW // 16
        CONP = np.zeros((128, WC), np.float16)
        o = 0
        CONP[:, o:o + NPC] = xT16; o += NPC
        CONP[:, o:o + L] = dmask[r]; o += L
        CONP[:, o:o + cfg.NPG] = gmask[r]; o += cfg.NPG
        for M in (LA, LB1, LB2, PERM16, SUMPERM, W2BD):
            CONP[:, o:o + 128] = M; o += 128
        CONP[:, o:o + 8] = np.asarray(W1, np.float16); o += 8
        CONP[0:32, o:o + 2] = np.asarray(W3, np.float16); o += 2
        SMLP = np.zeros((128, WS), np.float32)
        SMLP[0:8, 0] = np.asarray(b1, np.float32)
        SMLP[:, 1] = b2rep[:, 0]
        SMLP[0:2, 2] = np.asarray(b3, np.float32)
        SMLP[:, 3] = betavec[:, 0]
        SMLP[:, 4] = hnmask_np[:, 0]
        SMLP[:, 5] = onemask_np[:, 0]
        SMLP[0:32, 6:6 + cfg.G] = RC
        in_maps.append(dict(IDXP=IDXP, CONP=CONP, SMLP=SMLP))
    meta = dict(Rc=[int(v) for v in Rc], L=L, SI=SI, blocks=meta_blocks, KMAX=KMAX, SCATW=SCATW,
                WI=WI, WC=WC, WS=WS)
    return in_maps, meta


def build_kernel(cfg, meta, conv_reps=1):
    NC, NPC, NPG, G = cfg.NC, cfg.NPC, cfg.NPG, cfg.G
    Rc, L, SI = meta["Rc"], meta["L"], meta["SI"]
    ENDW, SUMW, MAXW, B = cfg.ENDW, cfg.SUMW, cfg.MAXW, cfg.B
    Roff = [0]
    for c in range(NC):
        Roff.append(Roff[-1] + Rc[c])
    RcMax = max(Rc)

    nc = bacc.Bacc("TRN2", target_bir_lowering=False, debug=False, num_devices=NC)

    def inp(name, shape, dt):
        return nc.dram_tensor(name, shape, dt, kind="ExternalInput").ap()

    KMAX = meta["KMAX"]
    blocks = meta["blocks"]
    SCATW = meta["SCATW"]
    IDXP = inp("IDXP", [128, meta["WI"]], I16)
    CONP = inp("CONP", [128, meta["WC"]], F16)
    SMLP = inp("SMLP", [128, meta["WS"]], F32)
    o = 0
    srcidx_d = IDXP[:, o:o + SI]; o += SI
    scatidx_d = IDXP[:, o:o + SCATW]; o += SCATW
    endsidx_d = IDXP[:, o:o + NC * (ENDW // 16)]; o += NC * (ENDW // 16)
    sumidx_d = IDXP[:, o:o + SUMW // 16]; o += SUMW // 16
    maxidx_d = IDXP[:, o:o + MAXW // 16]; o += MAXW // 16
    o = 0
    xT16 = CONP[:, o:o + NPC]; o += NPC
    dmask_d = CONP[:, o:o + L]; o += L
    gmask_d = CONP[:, o:o + NPG]; o += NPG
    LA_d = CONP[:, o:o + 128]; o += 128
    LB1_d = CONP[:, o:o + 128]; o += 128
    LB2_d = CONP[:, o:o + 128]; o += 128
    PERM16_d = CONP[:, o:o + 128]; o += 128
    SUMPERM_d = CONP[:, o:o + 128]; o += 128
    W2BD_d = CONP[:, o:o + 128]; o += 128
    W1_d = CONP[:, o:o + 8]; o += 8
    W3_d = CONP[0:32, o:o + 2]; o += 2
    b1_d = SMLP[0:8, 0:1]
    b2_d = SMLP[:, 1:2]
    b3_d = SMLP[0:2, 2:3]
    beta_d = SMLP[:, 3:4]
    hnmask_d = SMLP[:, 4:5]
    onemask_d = SMLP[:, 5:6]
    RC_d = SMLP[0:32, 6:6 + G]
    out_ext = nc.dram_tensor("out", [2, G], F32, kind="ExternalOutput").ap()


    contrib1 = nc.dram_tensor("contrib1", [16, NPC], F32)
    contrib2 = nc.dram_tensor("contrib2", [16, NPC], F32)
    glob1 = nc.dram_tensor("glob1", [NC * 16, NPC], F32, addr_space="Shared")
    glob2 = nc.dram_tensor("glob2", [NC * 16, NPC], F32, addr_space="Shared")
    cpool = nc.dram_tensor("cpool", [32, G], F32)
    gpool = nc.dram_tensor("gpool", [NC * 32, G], F32, addr_space="Shared")

    vg = [min(NPG, NPC - g * NPG) for g in range(8)]  # valid nodes per group

    from contextlib import ExitStack
    inp2 = inp
    with tile.TileContext(nc) as tc, ExitStack() as _es:
        sb = _es.enter_context(tc.tile_pool(name="sb", bufs=1))
        pp = _es.enter_context(tc.tile_pool(name="pp", bufs=2, space="PSUM"))

        # ---- load constants & index arrays
        LA = sb.tile([128, 128], F16); nc.sync.dma_start(out=LA[:], in_=LA_d[:, :])
        LB1 = sb.tile([128, 128], F16); nc.sync.dma_start(out=LB1[:], in_=LB1_d[:, :])
        LB2 = sb.tile([128, 128], F16); nc.sync.dma_start(out=LB2[:], in_=LB2_d[:, :])
        PERM = sb.tile([128, 128], F16); nc.sync.dma_start(out=PERM[:], in_=PERM16_d[:, :])
        SPERM = sb.tile([128, 128], F16); nc.sync.dma_start(out=SPERM[:], in_=SUMPERM_d[:, :])
        W1t = sb.tile([128, 8], F16); nc.sync.dma_start(out=W1t[:], in_=W1_d[:, :])
        b1t = sb.tile([8, 1], F32); nc.sync.dma_start(out=b1t[:], in_=b1_d[:, :])
        W2t = sb.tile([128, 128], F16); nc.sync.dma_start(out=W2t[:], in_=W2BD_d[:, :])
        b2t = sb.tile([128, 1], F32); nc.sync.dma_start(out=b2t[:], in_=b2_d[:, :])
        W3t = sb.tile([32, 2], F16); nc.sync.dma_start(out=W3t[:], in_=W3_d[:, :])
        b3t = sb.tile([2, 1], F32); nc.sync.dma_start(out=b3t[:], in_=b3_d[:, :])
        betat = sb.tile([128, 1], F32); nc.sync.dma_start(out=betat[:], in_=beta_d[:, :])
        gmask = sb.tile([128, NPG], F16); nc.sync.dma_start(out=gmask[:], in_=gmask_d[:, :])

        zcol = sb.tile([128, 1], F32); nc.vector.memset(zcol[:], 0.0)
        hnmask = sb.tile([128, 1], F32); nc.sync.dma_start(out=hnmask[:], in_=hnmask_d[:, :])
        onemask = sb.tile([128, 1], F32); nc.sync.dma_start(out=onemask[:], in_=onemask_d[:, :])
        betamask = sb.tile([128, 1], F32)
        nc.vector.tensor_tensor(out=betamask[:], in0=hnmask[:], in1=betat[:], op=ALU.mult)

        # ---- FC1: H1flat pieces -> H1G group-packed
        H1G = sb.tile([128, NPG], F32)
        nc.vector.memset(H1G[:], 0.0)
        np_pieces = [(o, min(512, NPC - o)) for o in range(0, NPC, 512)]
        for o, w in np_pieces:
            xts = sb.tile([128, 512], F16, name=f"xts_{o}", tag="xts", bufs=1)
            nc.sync.dma_start(out=xts[:, :w], in_=xT16[:, o:o + w])
            h1p = pp.tile([8, 512], F32, space="PSUM", name=f"h1p_{o}", tag="ssp", bufs=2)
            nc.tensor.matmul(out=h1p[:, :w], lhsT=W1t[:], rhs=xts[:, :w], start=True, stop=True)
            h1s = sb.tile([8, 512], F32, name=f"h1s_{o}", tag="h1s", bufs=2)
            nc.scalar.activation(h1s[:, :w], h1p[:, :w], ACT.Relu, bias=b1t[:])
            # DMA spans into H1G rows 16g+(0..8)
            a = o
            while a < o + w:
                g = a // NPG
                e = min(o + w, (g + 1) * NPG, NPC)
                nc.sync.dma_start(out=H1G[16 * g:16 * g + 8, a - g * NPG:e - g * NPG],
                                  in_=h1s[0:8, a - o:e - o])
                a = e

        # ---- helpers
        def build_table(HG, contrib, dsttab, with_beta):
            nonlocal_hn = None
            SQ16 = sb.tile([128, NPG], F16, tag="sq16", bufs=2)
            nc.vector.tensor_tensor(out=SQ16[:], in0=HG[:], in1=HG[:], op=ALU.mult)
            NRM = sb.tile([128, NPG], F32, tag="nrm", bufs=2)
            for o in range(0, NPG, 512):
                w = min(512, NPG - o)
                ssp = pp.tile([128, 512], F32, space="PSUM", name=f"ssp_{o}", tag="ssp", bufs=2)
                nc.tensor.matmul(out=ssp[:, :w], lhsT=SPERM[:], rhs=SQ16[:, o:o + w], start=True, stop=True)
                nc.scalar.activation(NRM[:, o:o + w], ssp[:, :w], ACT.Sqrt)
            nc.vector.tensor_scalar_max(NRM[:], NRM[:], 1e-6)
            nc.vector.reciprocal(NRM[:], NRM[:])
            HN = sb.tile([128, NPG], F32, tag="hn", bufs=2)
            nc.vector.tensor_tensor(out=HN[:], in0=HG[:], in1=NRM[:], op=ALU.mult)
            for g in range(8):
                v = vg[g]
                nc.sync.dma_start(out=contrib.ap()[0:8, g * NPG:g * NPG + v], in_=HN[16 * g:16 * g + 8, 0:v])
                nc.sync.dma_start(out=contrib.ap()[8:16, g * NPG:g * NPG + v], in_=HG[16 * g:16 * g + 8, 0:v])
            mvec = betamask if with_beta else hnmask
            nc.vector.tensor_scalar(out=dsttab[:, 0:NPG], in0=HN[:], scalar1=mvec[:], scalar2=onemask[:],
                                    op0=ALU.mult, op1=ALU.add)
            nc.vector.memset(dsttab[:, NPG:NPG + KMAX], 0.0)
            return HN

        def conv(glob, dsttab, NUMDEN, HGc, HNc, with_beta):
            nc.vector.memset(NUMDEN[:], 0.0)
            tab = sb.tile([128, NPC], F32)
            P = sb.tile([128, RcMax], F32)
            eidxc = sb.tile([128, ENDW // 16], I16, tag="eidxc", bufs=1)
            Gt = sb.tile([128, ENDW], F32)
            bi = -1
            for c in range(NC):
                for g in range(8):
                    nc.sync.dma_start(out=tab[16 * g:16 * g + 16, :], in_=glob.ap()[16 * c:16 * c + 16, :])
                o = Roff[c]
                nc.sync.dma_start(out=eidxc[:], in_=endsidx_d[:, c * (ENDW // 16):(c + 1) * (ENDW // 16)])
                SRCc = sb.tile([128, RcMax], F32, name=f"srcc_{c}", tag="SRCc", bufs=1)
                sidxcc = sb.tile([128, RcMax // 16], I16, name=f"sxc_{c}", tag="sidxcc", bufs=1)
                nc.sync.dma_start(out=sidxcc[:, :Rc[c] // 16], in_=srcidx_d[:, o // 16:(o + Rc[c]) // 16])
                nc.gpsimd.ap_gather(SRCc[:, :Rc[c]], tab[:], sidxcc[:, :Rc[c] // 16],
                                    channels=128, num_elems=NPC, d=1, num_idxs=Rc[c])
                prev_dst = None
                prev_w = 0
                for a in range(0, Rc[c], B):
                    bi += 1
                    _c2, _a2, W, j0, _K = blocks[bi]
                    assert _c2 == c and _a2 == a and W == min(B, Rc[c] - a)
                    PRODt = sb.tile([128, B], F16, name=f"prd_{c}_{a}", tag="PRODt", bufs=2)
                    DST16t = sb.tile([128, B], F16, name=f"d16_{c}_{a}", tag="DST16t", bufs=2)
                    Mt = sb.tile([128, B], F16, name=f"mt_{c}_{a}", tag="Mt", bufs=2)
                    SCT = sb.tile([128, B], F16, name=f"sct_{c}_{a}", tag="SCT", bufs=2)
                    scidc = sb.tile([128, KMAX], I16, name=f"sc_{c}_{a}", tag="scidc", bufs=2)
                    sdat = sb.tile([128, KMAX], F16, name=f"sd_{c}_{a}", tag="sdat", bufs=1)
                    nc.sync.dma_start(out=scidc[:], in_=scatidx_d[:, bi * KMAX:(bi + 1) * KMAX])
                    nc.sync.dma_start(out=Mt[:, :W], in_=dmask_d[:, o + a:o + a + W])
                    nc.vector.tensor_copy(out=sdat[:], in_=dsttab[:, j0:j0 + KMAX])
                    nc.gpsimd.local_scatter(SCT[:, :W], sdat[:], scidc[:],
                                            channels=128, num_elems=W, num_idxs=KMAX)
                    init = 0.0 if a == 0 else prev_dst[:, prev_w - 1:prev_w]
                    nc.vector.tensor_tensor_scan(DST16t[:, :W], Mt[:, :W], SCT[:, :W], init, ALU.mult, ALU.add)
                    prev_dst = DST16t
                    prev_w = W
                    nc.vector.tensor_tensor(out=PRODt[:, :W], in0=SRCc[:, a:a + W], in1=DST16t[:, :W], op=ALU.mult)
                    for po in range(0, W, 512):
                        pw = min(512, W - po)
                        alph = pp.tile([128, 512], F32, space="PSUM", name=f"al_{c}_{a}_{po}", tag="alph", bufs=3)
                        nc.tensor.matmul(out=alph[:, :pw], lhsT=LA[:], rhs=PRODt[:, po:po + pw], start=True, stop=True)
                        payp = sb.tile([128, 512], F32, name=f"pp_{c}_{a}_{po}", tag="payp", bufs=2)
                        nc.scalar.activation(payp[:, :pw], alph[:, :pw], ACT.Exp)
                        hp = pp.tile([128, 512], F32, space="PSUM", name=f"hp_{c}_{a}_{po}", tag="hp", bufs=3)
                        nc.tensor.matmul(out=hp[:, :pw], lhsT=LB1[:], rhs=PRODt[:, po:po + pw], start=True, stop=False)
                        nc.tensor.matmul(out=hp[:, :pw], lhsT=LB2[:], rhs=DST16t[:, po:po + pw], start=False, stop=True)
                        nc.vector.tensor_tensor(out=payp[:, :pw], in0=payp[:, :pw], in1=hp[:, :pw], op=ALU.mult)
                        init = 0.0 if (a + po) == 0 else P[:, a + po - 1:a + po]
                        nc.vector.tensor_tensor_scan(P[:, a + po:a + po + pw], payp[:, :pw],
                                                     zcol[:].to_broadcast([128, pw]), init, ALU.add, ALU.add)
                nc.gpsimd.ap_gather(Gt[:], P[:, :Rc[c]], eidxc[:],
                                    channels=128, num_elems=Rc[c], d=1, num_idxs=ENDW)
                nc.vector.tensor_tensor(out=NUMDEN[:], in0=NUMDEN[:], in1=Gt[:, 1:NPG + 1], op=ALU.add)
                nc.vector.tensor_tensor(out=NUMDEN[:], in0=NUMDEN[:], in1=Gt[:, 0:NPG], op=ALU.subtract)
            # analytic self-loop term: NUMDEN += exp(beta*||hn||^2) * (HG on h-rows | 1 on ones-rows)
            SQH = sb.tile([128, NPG], F16, tag="sq16", bufs=2)
            nc.vector.tensor_tensor(out=SQH[:], in0=HNc[:], in1=HNc[:], op=ALU.mult)
            SELFW = sb.tile([128, NPG], F32, tag="nrm", bufs=2)
            for o2 in range(0, NPG, 512):
                w2 = min(512, NPG - o2)
                qp = pp.tile([128, 512], F32, space="PSUM", name=f"qp_{o2}", tag="ssp", bufs=2)
                nc.tensor.matmul(out=qp[:, :w2], lhsT=SPERM[:], rhs=SQH[:, o2:o2 + w2], start=True, stop=True)
                if with_beta:
                    nc.scalar.activation(SELFW[:, o2:o2 + w2], qp[:, :w2], ACT.Exp, scale=betat[:])
                else:
                    nc.scalar.activation(SELFW[:, o2:o2 + w2], qp[:, :w2], ACT.Exp)
            SELFP = sb.tile([128, NPG], F16, tag="sq16", bufs=2)
            nc.vector.tensor_scalar(out=SELFP[:], in0=HGc[:], scalar1=hnmask[:], scalar2=onemask[:],
                                    op0=ALU.mult, op1=ALU.add)
            nc.vector.tensor_tensor(out=SELFP[:], in0=SELFP[:], in1=SELFW[:], op=ALU.mult)
            nc.vector.tensor_tensor(out=NUMDEN[:], in0=NUMDEN[:], in1=SELFP[:], op=ALU.add)

        def h_from_numden(ND):
            ND16 = sb.tile([128, NPG], F16, tag="sq16", bufs=2)
            nc.vector.tensor_copy(out=ND16[:], in_=ND[:])
            SREP = sb.tile([128, NPG], F32, tag="nrm", bufs=2)
            for o in range(0, NPG, 512):
                w = min(512, NPG - o)
                srp = pp.tile([128, 512], F32, space="PSUM", name=f"srp_{o}", tag="ssp", bufs=2)
                nc.tensor.matmul(out=srp[:, :w], lhsT=PERM[:], rhs=ND16[:, o:o + w], start=True, stop=True)
                nc.vector.tensor_copy(out=SREP[:, o:o + w], in_=srp[:, :w])
            nc.vector.tensor_scalar_max(SREP[:], SREP[:], 1e-12)
            nc.vector.reciprocal(SREP[:], SREP[:])
            nc.vector.tensor_tensor(out=ND[:], in0=ND[:], in1=SREP[:], op=ALU.mult)

        # ---- pipeline
        dsttab = sb.tile([128, NPG + KMAX], F16)
        NUMDEN = sb.tile([128, NPG], F32)
        HN1 = build_table(H1G, contrib1, dsttab, with_beta=False)
        nc.gpsimd.collective_compute("AllGather", ALU.bypass, ins=[contrib1.ap().opt()],
                                     outs=[glob1.ap().opt()], replica_groups=[list(range(NC))])
        for _cr in range(conv_reps):
            conv(glob1, dsttab, NUMDEN, H1G, HN1, with_beta=False)
        h_from_numden(NUMDEN)
        H2G = sb.tile([128, NPG], F32)
        nc.vector.tensor_scalar(out=H2G[:], in0=NUMDEN[:], scalar1=hnmask[:], scalar2=None, op0=ALU.mult)
        HN2 = build_table(H2G, contrib2, dsttab, with_beta=True)
        nc.gpsimd.collective_compute("AllGather", ALU.bypass, ins=[contrib2.ap().opt()],
                                     outs=[glob2.ap().opt()], replica_groups=[list(range(NC))])
        conv(glob2, dsttab, NUMDEN, H2G, HN2, with_beta=True)
        h_from_numden(NUMDEN)

        # ---- FC2 + relu (block-diagonal W2)
        H2n16 = sb.tile([128, NPG], F16, tag="sq16", bufs=2)
        nc.vector.tensor_copy(out=H2n16[:], in_=NUMDEN[:])
        H3G = sb.tile([128, NPG], F32, tag="hn", bufs=2)
        for o in range(0, NPG, 512):
            w = min(512, NPG - o)
            f2p = pp.tile([128, 512], F32, space="PSUM", name=f"f2p_{o}", tag="ssp", bufs=2)
            nc.tensor.matmul(out=f2p[:, :w], lhsT=W2t[:], rhs=H2n16[:, o:o + w], start=True, stop=True)
            nc.scalar.activation(H3G[:, o:o + w], f2p[:, :w], ACT.Relu, bias=b2t[:])

        # ---- pooling: scans + boundary gathers
        SUMB = sb.tile([128, NPG + 16], F32)
        MAXB = sb.tile([128, NPG + 16], F32)
        nc.vector.memset(SUMB[:], 0.0)
        nc.vector.memset(MAXB[:], 0.0)
        nc.vector.tensor_tensor_scan(SUMB[:, 4:4 + NPG], H3G[:], zcol[:].to_broadcast([128, NPG]), 0.0, ALU.add, ALU.add)
        nc.vector.tensor_tensor_scan(MAXB[:, 4:4 + NPG], gmask[:], H3G[:], 0.0, ALU.mult, ALU.max)
        sidx = sb.tile([128, SUMW // 16], I16); nc.sync.dma_start(out=sidx[:], in_=sumidx_d[:, :])
        midx = sb.tile([128, MAXW // 16], I16); nc.sync.dma_start(out=midx[:], in_=maxidx_d[:, :])
        GS = sb.tile([128, SUMW], F32)
        GM = sb.tile([128, MAXW], F32)
        nc.gpsimd.ap_gather(GS[:], SUMB[:], sidx[:], channels=128, num_elems=NPG + 16, d=1, num_idxs=SUMW)
        nc.gpsimd.ap_gather(GM[:], MAXB[:], midx[:], channels=128, num_elems=NPG + 16, d=1, num_idxs=MAXW)
        SUMP = sb.tile([128, G], F32)
        nc.vector.tensor_tensor(out=SUMP[:], in0=GS[:, 1:G + 1], in1=GS[:, 0:G], op=ALU.subtract)
        # restack 8 groups -> [16, G, 8], reduce
        PMs = sb.tile([16, G, 8], F32)
        PSs = sb.tile([16, G, 8], F32)
        for g in range(8):
            nc.sync.dma_start(out=PMs[:, :, g:g + 1], in_=GM[16 * g:16 * g + 16, 0:G, None])
            nc.sync.dma_start(out=PSs[:, :, g:g + 1], in_=SUMP[16 * g:16 * g + 16, :, None])
        PM = sb.tile([16, G], F32)
        PS = sb.tile([16, G], F32)
        nc.vector.tensor_reduce(out=PM[:], in_=PMs[:], axis=mybir.AxisListType.X, op=ALU.max)
        nc.vector.tensor_reduce(out=PS[:], in_=PSs[:], axis=mybir.AxisListType.X, op=ALU.add)
        nc.sync.dma_start(out=cpool.ap()[0:16, :], in_=PM[:])
        nc.sync.dma_start(out=cpool.ap()[16:32, :], in_=PS[:])
        nc.gpsimd.collective_compute("AllGather", ALU.bypass, ins=[cpool.ap().opt()],
                                     outs=[gpool.ap().opt()], replica_groups=[list(range(NC))])
        GPLM = sb.tile([16, G, NC], F32)
        GPLS = sb.tile([16, G, NC], F32)
        gp4 = gpool.ap().rearrange("(r two p) f -> two p f r", two=2, p=16)
        nc.sync.dma_start(out=GPLM[:], in_=gp4[0])
        nc.sync.dma_start(out=GPLS[:], in_=gp4[1])
        GMPt = sb.tile([16, G], F32)
        GAPt = sb.tile([16, G], F32)
        nc.vector.tensor_reduce(out=GMPt[:], in_=GPLM[:], axis=mybir.AxisListType.X, op=ALU.max)
        nc.vector.tensor_reduce(out=GAPt[:], in_=GPLS[:], axis=mybir.AxisListType.X, op=ALU.add)
        GPOOL2 = sb.tile([32, G], F32)
        nc.sync.dma_start(out=GPOOL2[0:16, :], in_=GMPt[:])
        nc.sync.dma_start(out=GPOOL2[16:32, :], in_=GAPt[:])
        RCt = sb.tile([32, G], F32); nc.sync.dma_start(out=RCt[:], in_=RC_d[:, :])
        nc.vector.tensor_tensor(out=GPOOL2[:], in0=GPOOL2[:], in1=RCt[:], op=ALU.mult)
        PF16 = sb.tile([32, G], F16)
        nc.vector.tensor_copy(out=PF16[:], in_=GPOOL2[:])
        f3p = pp.tile([2, G], F32, space="PSUM", name="f3p", tag="alph", bufs=3)
        nc.tensor.matmul(out=f3p[:], lhsT=W3t[:], rhs=PF16[:], start=True, stop=True)
        OUTT = sb.tile([2, G], F32)
        nc.vector.tensor_scalar(out=OUTT[:], in0=f3p[:], scalar1=b3t[:], scalar2=None, op0=ALU.add)
        nc.sync.dma_start(out=out_ext[:, :], in_=OUTT[:])

    nc.compile()
    return nc


class SpmdRunner:

    def __init__(self, nc, n_cores=8):
        install_neuronx_cc_hook()
        self.nc = nc
        self.n_cores = n_cores
        assert nc.dbg_addr is None or not nc.dbg_callbacks
        partition_name = nc.partition_id_tensor.name if nc.partition_id_tensor else None
        in_names, out_names, out_avals, zero_outs = [], [], [], []
        for alloc in nc.m.functions[0].allocations:
            if not isinstance(alloc, mybir.MemoryLocationSet):
                continue
            name = alloc.memorylocations[0].name
            if alloc.kind == "ExternalInput":
                if name != partition_name:
                    in_names.append(name)
            elif alloc.kind == "ExternalOutput":
                out_names.append(name)
                shape = tuple(alloc.tensor_shape)
                dtype = mybir.dt.np(alloc.dtype)
                out_avals.append(jax.core.ShapedArray(shape, dtype))
                zero_outs.append(np.zeros(shape, dtype))
        self.dbg_name = nc.dbg_addr.name if nc.dbg_addr is not None else None
        if self.dbg_name is not None and self.dbg_name in in_names:
            pass  # keep; we must supply zeros
        self.in_names = list(in_names)
        self.out_names = out_names
        self.out_avals = out_avals
        self.zero_outs = zero_outs
        n_params = len(in_names)
        n_outs = len(out_avals)
        all_in_names = list(in_names) + list(out_names)
        if partition_name is not None:
            all_in_names.append(partition_name)
        self.partition_name = partition_name

        def _body(*args):
            operands = list(args)
            if partition_name is not None:
                operands.append(partition_id_tensor())
            outs = _bass_exec_p.bind(
                *operands,
                out_avals=tuple(out_avals),
                in_names=tuple(all_in_names),
                out_names=tuple(out_names),
                lowering_input_output_aliases=(),
                sim_require_finite=True,
                sim_require_nnan=True,
                nc=nc,
            )
            return tuple(outs)

        devices = jax.devices()[: n_cores]
        self.mesh = Mesh(np.asarray(devices), ("core",))
        in_specs = (PartitionSpec("core"),) * (n_params + n_outs)
        out_specs = (PartitionSpec("core"),) * n_outs
        donate = tuple(range(n_params, n_params + n_outs))
        self.fn = jax.jit(
            shard_map(_body, mesh=self.mesh, in_specs=in_specs, out_specs=out_specs, check_rep=False),
            donate_argnums=donate, keep_unused=True,
        )

    def prep_inputs(self, in_maps):
        """in_maps: list of dicts per core. Returns device-put concatenated inputs."""
        n = self.n_cores
        if self.dbg_name is not None:
            in_maps = [{**m, self.dbg_name: np.zeros((1, 2), np.uint32)} for m in in_maps]
        concat = [np.concatenate([np.asarray(in_maps[c][nm]) for c in range(n)], axis=0) for nm in self.in_names]
        sh = jax.sharding.NamedSharding(self.mesh, PartitionSpec("core"))
        return [jax.device_put(a, sh) for a in concat]

    def make_zeros(self):
        sh = jax.sharding.NamedSharding(self.mesh, PartitionSpec("core"))
        return [jax.device_put(np.zeros((self.n_cores * z.shape[0], *z.shape[1:]), z.dtype), sh) for z in self.zero_outs]

    def run(self, dev_inputs):
        outs = self.fn(*dev_inputs, *self.make_zeros())
        jax.block_until_ready(outs)
        return outs

    def results(self, outs):
        res = []
        for c in range(self.n_cores):
            d = {}
            for i, name in enumerate(self.out_names):
                d[name] = np.asarray(outs[i]).reshape(self.n_cores, *self.out_avals[i].shape)[c]
            res.append(d)
        return res

    def bench(self, dev_inputs, iters=10, warmup=2):
        for _ in range(warmup):
            self.run(dev_inputs)
        ts = []
        for _ in range(iters):
            t0 = time.perf_counter()
            self.run(dev_inputs)
            ts.append(time.perf_counter() - t0)
        return ts

    def chain(self, dev_inputs, iters, outs=None):
        """Run `iters` chained executions: iteration k's donated output
        buffers are iteration k-1's outputs, so the executions queue
        back-to-back on the NeuronCores with one client sync at the end.
        Returns wall time and the final outputs."""
        if outs is None:
            outs = self.make_zeros()
        t0 = time.perf_counter()
        for _ in range(iters):
            outs = list(self.fn(*dev_inputs, *outs))
        jax.block_until_ready(outs)
        return time.perf_counter() - t0, outs

    def bench_chained(self, dev_inputs, k1=10, k2=60, reps=3):
        """Per-execution device time via the two-point slope
        (T(k2)-T(k1))/(k2-k1), which cancels the constant client<->device
        round-trip latency that a single blocking run would measure.
        Returns the min slope over `reps` repetitions, in seconds."""
        _, outs = self.chain(dev_inputs, 2)  # warmup
        best = float("inf")
        for _ in range(reps):
            t1, outs = self.chain(dev_inputs, k1, outs)
            t2, outs = self.chain(dev_inputs, k2, outs)
            best = min(best, (t2 - t1) / (k2 - k1))
        return best


_CACHE = {}


def kernel(**inputs):
    x = np.asarray(inputs["x"], np.float32)
    edge_index = np.asarray(inputs["edge_index"])
    batch = np.asarray(inputs["batch"])
    cfg = Cfg(N=x.shape[0], E=edge_index.shape[1], G=64, NC=8)
    im, meta = preprocess(cfg, x, edge_index, batch, inputs["W1"], inputs["b1"],
                          inputs["beta2"], inputs["W2"], inputs["b2"], inputs["W3"], inputs["b3"])
    key = (cfg.N, cfg.E, tuple(meta["Rc"]), meta["SCATW"])
    if key not in _CACHE:
        nc = build_kernel(cfg, meta)
        _CACHE[key] = SpmdRunner(nc, n_cores=cfg.NC)
    r = _CACHE[key]
    din = r.prep_inputs(im)
    t0 = time.perf_counter()
    outs = r.run(din)
    wall = time.perf_counter() - t0
    res = r.results(outs)
    kernel.last_wall_s = wall
    kernel.runner = r
    kernel.dev_inputs = din
    out = res[0]["out"].T.astype(np.float32)  # [G, 2]
    return np.ascontiguousarray(out)



# revision 8
# speedup vs baseline: 1.0366x; 1.0366x over previous
"""AGNN message-passing kernel for 8 TRN2 NeuronCores (Bass/Tile).

Sharding: nodes dst-sharded 8 ways; edges colocated with their target node
(sorted per dst-group / src-chunk / dst).  Per conv layer: source features
gathered Q7-group-parallel with ap_gather from fp32 [hn|h] feature-major
chunk tables; dst features expanded by a boundary-reset DVE prefix scan fed
by GPSIMD local_scatter (no per-edge dst gather); self-loop contributions
added in closed form (exp(beta*||hn||^2)*[h|1]) so they need no edge slots;
attention = replicated-alpha matmuls + ScalarE exp; scatter-softmax segment
sums via DVE cumsum + ap_gather boundary diffs; tables exchanged with
AllGather; global max/avg pool via masked max-scan / cumsum + AllGather.
Host side does index/layout preprocessing only.
"""
import time
import numpy as np
import jax
from jax.sharding import Mesh, PartitionSpec
from jax.experimental.shard_map import shard_map
from concourse import bass, mybir, bacc
from concourse import bass2jax
from concourse.bass2jax import _bass_exec_p, install_neuronx_cc_hook, partition_id_tensor
import concourse.tile as tile

F16 = mybir.dt.float16
F32 = mybir.dt.float32
I16 = mybir.dt.int16
ALU = mybir.AluOpType
ACT = mybir.ActivationFunctionType


def r16(x):
    return ((x + 15) // 16) * 16


class Cfg:
    def __init__(self, N=100000, E=3200000, D=128, G=64, NC=8):
        self.N, self.E, self.D, self.G, self.NC = N, E, D, G, NC
        self.NPC = N // NC            # nodes per core
        self.NPG = (self.NPC + 7) // 8  # nodes per Q7 group
        self.ENDW = r16(self.NPG + 1)   # ends list width per chunk
        self.SUMW = r16(G + 1)          # sum-pool idx width
        self.MAXW = r16(G)              # max-pool idx width
        self.B = 1024                   # edge block size


def preprocess(cfg, x, edge_index, batch, W1, b1, beta2, W2, b2, W3, b3):
    """Pure index/layout preprocessing. Returns (in_maps list per core, meta dict)."""
    N, NC, NPC, NPG = cfg.N, cfg.NC, cfg.NPC, cfg.NPG
    src = np.asarray(edge_index[0], dtype=np.int64)
    dst = np.asarray(edge_index[1], dtype=np.int64)
    batch = np.asarray(batch, dtype=np.int64)

    core = dst // NPC
    dstl = dst - core * NPC
    grp = dstl // NPG
    chunk = src // NPC
    order = np.lexsort((dst, chunk, grp, core))
    src, dst, core, dstl, grp, chunk = (a[order] for a in (src, dst, core, dstl, grp, chunk))
    srcl = src - chunk * NPC

    # counts per (core, grp, chunk)
    key = (core * 8 + grp) * NC + chunk
    cnt = np.bincount(key, minlength=NC * 8 * NC).reshape(NC, 8, NC)
    Rc = np.array([r16(cnt[:, :, c].max() + 4) for c in range(NC)], dtype=np.int64)
    Roff = np.concatenate([[0], np.cumsum(Rc)])
    L = int(Roff[-1])          # per-group edge array length (uniform)
    SI = L // 16

    starts = np.zeros(NC * 8 * NC, dtype=np.int64)
    np.cumsum(cnt.reshape(-1)[:-1], out=starts[1:])
    # position within (core,grp,chunk) block
    within = np.arange(len(src)) - starts[key]
    pos = Roff[chunk] + 4 + within  # position in the group's edge array

    part = (grp * 16 + pos % 16).astype(np.int64)
    col = (pos // 16).astype(np.int64)

    srcidx = np.zeros((NC, 128, SI), dtype=np.int16)
    srcidx[core, part, col] = srcl.astype(np.int16)

    # --- dst-side scan-expand metadata ---
    B = cfg.B
    dmask = np.ones((NC, 128, L), dtype=np.float16)
    # run starts: first edge of each (core,grp,chunk,node) run
    gl = dstl - grp * NPG  # group-local node
    first = np.ones(len(src), dtype=bool)
    first[1:] = (key[1:] != key[:-1]) | (gl[1:] != gl[:-1])
    sp = pos[first]
    dmask[core[first][:, None], (grp[first] * 16)[:, None] + np.arange(16)[None, :], sp[:, None]] = 0.0
    # per (chunk, block): node span [j0, j0+K) uniform across cores/groups; scatter idx lists
    blocks = []
    for c in range(NC):
        for a in range(0, int(Rc[c]), B):
            blocks.append((c, a, min(B, int(Rc[c]) - a)))
    # node range per (core,grp,chunk,block): nodes of runs starting in block, plus carry-in node
    scat_parts = []
    KBLK = []
    # precompute per-edge block id within chunk
    blkid = within + 4  # position within chunk region
    for (c, a, W) in blocks:
        sel = (chunk == c) & (blkid >= a) & (blkid < a + W)
        j0 = 0; j1 = 1
        if sel.any():
            j0 = int(gl[sel].min()); j1 = int(gl[sel].max()) + 1
        K = max(j1 - j0, 1)
        KBLK.append((c, a, W, j0, K))
    KMAX = max(k[4] for k in KBLK)
    KMAX = ((KMAX + 15) // 16) * 16
    SCATW = KMAX * len(KBLK)
    scatidx = np.full((NC, 128, SCATW), -1, dtype=np.int16)
    sc_chunk = chunk[first]; sc_core = core[first]; sc_grp = grp[first]; sc_gl = gl[first]
    sc_blk = blkid[first] // B  # block index within chunk
    nblk_per_chunk = [int((Rc[c] + B - 1) // B) for c in range(NC)]
    cum_blk = np.concatenate([[0], np.cumsum(nblk_per_chunk)])
    j0_arr = np.zeros(len(KBLK), np.int64)
    for bi, (c, a, W, j0, K) in enumerate(KBLK):
        j0_arr[bi] = j0
    gb = cum_blk[sc_chunk] + sc_blk  # global block id per run-start
    rel = sc_gl - j0_arr[gb]
    okm = (rel >= 0) & (rel < KMAX)
    scatidx[sc_core[okm][:, None], (sc_grp[okm] * 16)[:, None] + np.arange(16)[None, :],
            (gb[okm] * KMAX + rel[okm])[:, None]] = ((blkid[first][okm] - (sc_blk[okm] * B))[:, None]).astype(np.int16)
    meta_blocks = [(c, a, W, int(j0_arr[bi]), KMAX) for bi, (c, a, W, j0, K) in enumerate(KBLK)]

    # ends per (core, grp, chunk): [3, 3+cum(0), ..., 3+cum(NPG-1)] padded to ENDW
    # cum over nodes of the group within this chunk
    nodecnt = np.bincount((core * 8 + grp) * (NC * NPG) + chunk * NPG + (dstl - grp * NPG),
                          minlength=NC * 8 * NC * NPG).reshape(NC, 8, NC, NPG)
    cum = np.cumsum(nodecnt, axis=3)
    ends = np.zeros((NC, 8, NC, cfg.ENDW), dtype=np.int64)
    ends[:, :, :, 0] = 3
    ends[:, :, :, 1:NPG + 1] = 3 + cum
    ends[:, :, :, NPG + 1:] = ends[:, :, :, NPG:NPG + 1]
    endsidx = np.zeros((NC, 128, NC * (cfg.ENDW // 16)), dtype=np.int16)
    for g in range(8):
        for c in range(NC):
            e = ends[:, g, c, :]  # [NC, ENDW]
            w = e.reshape(NC, cfg.ENDW // 16, 16).transpose(0, 2, 1)
            endsidx[:, 16 * g:16 * g + 16, c * (cfg.ENDW // 16):(c + 1) * (cfg.ENDW // 16)] = w

    # pooling: node -> graph, per (core, grp): counts per graph
    gnode = batch  # [N]
    nodecore = np.arange(N) // NPC
    nodegrp = (np.arange(N) % NPC) // NPG
    pk = (nodecore * 8 + nodegrp) * cfg.G + gnode
    pcnt = np.bincount(pk, minlength=NC * 8 * cfg.G).reshape(NC, 8, cfg.G)
    pcum = np.cumsum(pcnt, axis=2)
    sumends = np.zeros((NC, 8, cfg.SUMW), dtype=np.int64)
    sumends[:, :, 0] = 3
    sumends[:, :, 1:cfg.G + 1] = 3 + pcum
    sumends[:, :, cfg.G + 1:] = sumends[:, :, cfg.G:cfg.G + 1]
    maxends = np.where(pcnt > 0, 3 + pcum, 3)  # absent -> sentinel 3 (value 0)
    sumidx = np.zeros((NC, 128, cfg.SUMW // 16), dtype=np.int16)
    maxidx = np.zeros((NC, 128, cfg.MAXW // 16), dtype=np.int16)
    for g in range(8):
        sumidx[:, 16 * g:16 * g + 16, :] = sumends[:, g].reshape(NC, cfg.SUMW // 16, 16).transpose(0, 2, 1)
        me = np.zeros((NC, cfg.MAXW), dtype=np.int64)
        me[:, :cfg.G] = maxends[:, g]
        maxidx[:, 16 * g:16 * g + 16, :] = me.reshape(NC, cfg.MAXW // 16, 16).transpose(0, 2, 1)

    # graph-start mask per (core, grp): zeros at first col of each present graph, and col 0
    gmask = np.ones((NC, 128, NPG), dtype=np.float16)
    gstart = np.zeros((NC, 8, NPG), dtype=bool)
    gstart[:, :, 0] = True
    prev = np.concatenate([np.zeros((NC, 8, 1), np.int64), pcum[:, :, :-1]], axis=2)
    for r in range(NC):
        for g in range(8):
            s = prev[r, g][pcnt[r, g] > 0]
            s = s[s < NPG]
            gstart[r, g, s] = True
    for g in range(8):
        gmask[:, 16 * g:16 * g + 16, :] = np.where(gstart[:, g], 0.0, 1.0)[:, None, :]

    gcnt = np.maximum(np.bincount(batch, minlength=cfg.G).astype(np.float32), 1.0)
    RC = np.ones((32, cfg.G), dtype=np.float32)
    RC[16:32, :] = (1.0 / gcnt)[None, :]

    # constant matrices
    LA = np.zeros((128, 128), np.float16)
    LB1 = np.zeros((128, 128), np.float16)
    LB2 = np.zeros((128, 128), np.float16)
    PERM16 = np.zeros((128, 128), np.float16)
    SUMPERM = np.zeros((128, 128), np.float16)
    for g in range(8):
        b = 16 * g
        LA[b:b + 8, b:b + 16] = 1.0
        for r in range(8):
            LB1[b + 8 + r, b + r] = 1.0
            LB2[b + 8 + r, b + 8 + r] = 1.0
        PERM16[b + 8, b:b + 16] = 1.0
        SUMPERM[b:b + 8, b:b + 16] = 1.0
    W2BD = np.zeros((128, 128), np.float16)
    b2rep = np.zeros((128, 1), np.float32)
    for g in range(8):
        W2BD[16 * g:16 * g + 8, 16 * g:16 * g + 16] = np.asarray(W1, np.float16)[:8, :16] if False else np.asarray(W2, np.float16)
        b2rep[16 * g:16 * g + 16, 0] = np.asarray(b2, np.float32)
    betavec = np.full((128, 1), float(np.asarray(beta2)), np.float32)
    hnmask_np = np.zeros((128, 1), np.float32)
    onemask_np = np.ones((128, 1), np.float32)
    for g in range(8):
        hnmask_np[16 * g:16 * g + 8, 0] = 1.0
        onemask_np[16 * g:16 * g + 8, 0] = 0.0

    # ---- pack everything into 3 input tensors (NEFF launch pays ~45us per
    # input tensor, so input count dominates the fixed per-run overhead) ----
    SCATW = scatidx.shape[2]
    WI = r16(SI + SCATW + NC * (cfg.ENDW // 16) + cfg.SUMW // 16 + cfg.MAXW // 16)
    WC = r16(NPC + L + cfg.NPG + 6 * 128 + 8 + 2)
    WS = r16(6 + cfg.G)

    xf = np.asarray(x, np.float32)
    in_maps = []
    for r in range(NC):
        xT16 = np.ascontiguousarray(xf[r * NPC:(r + 1) * NPC, :].T).astype(np.float16)
        IDXP = np.zeros((128, WI), np.int16)
        o = 0
        IDXP[:, o:o + SI] = srcidx[r]; o += SI
        IDXP[:, o:o + SCATW] = scatidx[r]; o += SCATW
        IDXP[:, o:o + NC * (cfg.ENDW // 16)] = endsidx[r]; o += NC * (cfg.ENDW // 16)
        IDXP[:, o:o + cfg.SUMW // 16] = sumidx[r]; o += cfg.SUMW // 16
        IDXP[:, o:o + cfg.MAXW // 16] = maxidx[r]; o += cfg.MAXW // 16
        CONP = np.zeros((128, WC), np.float16)
        o = 0
        CONP[:, o:o + NPC] = xT16; o += NPC
        CONP[:, o:o + L] = dmask[r]; o += L
        CONP[:, o:o + cfg.NPG] = gmask[r]; o += cfg.NPG
        for M in (LA, LB1, LB2, PERM16, SUMPERM, W2BD):
            CONP[:, o:o + 128] = M; o += 128
        CONP[:, o:o + 8] = np.asarray(W1, np.float16); o += 8
        CONP[0:32, o:o + 2] = np.asarray(W3, np.float16); o += 2
        SMLP = np.zeros((128, WS), np.float32)
        SMLP[0:8, 0] = np.asarray(b1, np.float32)
        SMLP[:, 1] = b2rep[:, 0]
        SMLP[0:2, 2] = np.asarray(b3, np.float32)
        SMLP[:, 3] = betavec[:, 0]
        SMLP[:, 4] = hnmask_np[:, 0]
        SMLP[:, 5] = onemask_np[:, 0]
        SMLP[0:32, 6:6 + cfg.G] = RC
        in_maps.append(dict(IDXP=IDXP, CONP=CONP, SMLP=SMLP))
    meta = dict(Rc=[int(v) for v in Rc], L=L, SI=SI, blocks=meta_blocks, KMAX=KMAX, SCATW=SCATW,
                WI=WI, WC=WC, WS=WS)
    return in_maps, meta


def build_kernel(cfg, meta, conv_reps=1):
    NC, NPC, NPG, G = cfg.NC, cfg.NPC, cfg.NPG, cfg.G
    Rc, L, SI = meta["Rc"], meta["L"], meta["SI"]
    ENDW, SUMW, MAXW, B = cfg.ENDW, cfg.SUMW, cfg.MAXW, cfg.B
    Roff = [0]
    for c in range(NC):
        Roff.append(Roff[-1] + Rc[c])
    RcMax = max(Rc)

    nc = bacc.Bacc("TRN2", target_bir_lowering=False, debug=False, num_devices=NC)

    def inp(name, shape, dt):
        return nc.dram_tensor(name, shape, dt, kind="ExternalInput").ap()

    KMAX = meta["KMAX"]
    blocks = meta["blocks"]
    SCATW = meta["SCATW"]
    IDXP = inp("IDXP", [128, meta["WI"]], I16)
    CONP = inp("CONP", [128, meta["WC"]], F16)
    SMLP = inp("SMLP", [128, meta["WS"]], F32)
    o = 0
    srcidx_d = IDXP[:, o:o + SI]; o += SI
    scatidx_d = IDXP[:, o:o + SCATW]; o += SCATW
    endsidx_d = IDXP[:, o:o + NC * (ENDW // 16)]; o += NC * (ENDW // 16)
    sumidx_d = IDXP[:, o:o + SUMW // 16]; o += SUMW // 16
    maxidx_d = IDXP[:, o:o + MAXW // 16]; o += MAXW // 16
    o = 0
    xT16 = CONP[:, o:o + NPC]; o += NPC
    dmask_d = CONP[:, o:o + L]; o += L
    gmask_d = CONP[:, o:o + NPG]; o += NPG
    LA_d = CONP[:, o:o + 128]; o += 128
    LB1_d = CONP[:, o:o + 128]; o += 128
    LB2_d = CONP[:, o:o + 128]; o += 128
    PERM16_d = CONP[:, o:o + 128]; o += 128
    SUMPERM_d = CONP[:, o:o + 128]; o += 128
    W2BD_d = CONP[:, o:o + 128]; o += 128
    W1_d = CONP[:, o:o + 8]; o += 8
    W3_d = CONP[0:32, o:o + 2]; o += 2
    b1_d = SMLP[0:8, 0:1]
    b2_d = SMLP[:, 1:2]
    b3_d = SMLP[0:2, 2:3]
    beta_d = SMLP[:, 3:4]
    hnmask_d = SMLP[:, 4:5]
    onemask_d = SMLP[:, 5:6]
    RC_d = SMLP[0:32, 6:6 + G]
    out_ext = nc.dram_tensor("out", [2, G], F32, kind="ExternalOutput").ap()


    contrib1 = nc.dram_tensor("contrib1", [16, NPC], F32)
    contrib2 = nc.dram_tensor("contrib2", [16, NPC], F32)
    glob1 = nc.dram_tensor("glob1", [NC * 16, NPC], F32, addr_space="Shared")
    glob2 = nc.dram_tensor("glob2", [NC * 16, NPC], F32, addr_space="Shared")
    cpool = nc.dram_tensor("cpool", [32, G], F32)
    gpool = nc.dram_tensor("gpool", [NC * 32, G], F32, addr_space="Shared")

    vg = [min(NPG, NPC - g * NPG) for g in range(8)]  # valid nodes per group

    from contextlib import ExitStack
    inp2 = inp
    with tile.TileContext(nc) as tc, ExitStack() as _es:
        sb = _es.enter_context(tc.tile_pool(name="sb", bufs=1))
        pp = _es.enter_context(tc.tile_pool(name="pp", bufs=2, space="PSUM"))

        # ---- load constants & index arrays
        LA = sb.tile([128, 128], F16); nc.sync.dma_start(out=LA[:], in_=LA_d[:, :])
        LB1 = sb.tile([128, 128], F16); nc.sync.dma_start(out=LB1[:], in_=LB1_d[:, :])
        LB2 = sb.tile([128, 128], F16); nc.sync.dma_start(out=LB2[:], in_=LB2_d[:, :])
        PERM = sb.tile([128, 128], F16); nc.sync.dma_start(out=PERM[:], in_=PERM16_d[:, :])
        SPERM = sb.tile([128, 128], F16); nc.sync.dma_start(out=SPERM[:], in_=SUMPERM_d[:, :])
        W1t = sb.tile([128, 8], F16); nc.sync.dma_start(out=W1t[:], in_=W1_d[:, :])
        b1t = sb.tile([8, 1], F32); nc.sync.dma_start(out=b1t[:], in_=b1_d[:, :])
        W2t = sb.tile([128, 128], F16); nc.sync.dma_start(out=W2t[:], in_=W2BD_d[:, :])
        b2t = sb.tile([128, 1], F32); nc.sync.dma_start(out=b2t[:], in_=b2_d[:, :])
        W3t = sb.tile([32, 2], F16); nc.sync.dma_start(out=W3t[:], in_=W3_d[:, :])
        b3t = sb.tile([2, 1], F32); nc.sync.dma_start(out=b3t[:], in_=b3_d[:, :])
        betat = sb.tile([128, 1], F32); nc.sync.dma_start(out=betat[:], in_=beta_d[:, :])
        gmask = sb.tile([128, NPG], F16); nc.sync.dma_start(out=gmask[:], in_=gmask_d[:, :])

        zcol = sb.tile([128, 1], F32); nc.vector.memset(zcol[:], 0.0)
        hnmask = sb.tile([128, 1], F32); nc.sync.dma_start(out=hnmask[:], in_=hnmask_d[:, :])
        onemask = sb.tile([128, 1], F32); nc.sync.dma_start(out=onemask[:], in_=onemask_d[:, :])
        betamask = sb.tile([128, 1], F32)
        nc.vector.tensor_tensor(out=betamask[:], in0=hnmask[:], in1=betat[:], op=ALU.mult)

        # ---- FC1: H1flat pieces -> H1G group-packed
        H1G = sb.tile([128, NPG], F32)
        nc.vector.memset(H1G[:], 0.0)
        np_pieces = [(o, min(512, NPC - o)) for o in range(0, NPC, 512)]
        for o, w in np_pieces:
            xts = sb.tile([128, 512], F16, name=f"xts_{o}", tag="xts", bufs=2)
            nc.sync.dma_start(out=xts[:, :w], in_=xT16[:, o:o + w])
            h1p = pp.tile([8, 512], F32, space="PSUM", name=f"h1p_{o}", tag="ssp", bufs=2)
            nc.tensor.matmul(out=h1p[:, :w], lhsT=W1t[:], rhs=xts[:, :w], start=True, stop=True)
            h1s = sb.tile([8, 512], F32, name=f"h1s_{o}", tag="h1s", bufs=2)
            nc.scalar.activation(h1s[:, :w], h1p[:, :w], ACT.Relu, bias=b1t[:])
            # DMA spans into H1G rows 16g+(0..8)
            a = o
            while a < o + w:
                g = a // NPG
                e = min(o + w, (g + 1) * NPG, NPC)
                nc.sync.dma_start(out=H1G[16 * g:16 * g + 8, a - g * NPG:e - g * NPG],
                                  in_=h1s[0:8, a - o:e - o])
                a = e

        # ---- helpers
        def build_table(HG, contrib, dsttab, with_beta):
            nonlocal_hn = None
            SQ16 = sb.tile([128, NPG], F16, tag="sq16", bufs=2)
            nc.vector.tensor_tensor(out=SQ16[:], in0=HG[:], in1=HG[:], op=ALU.mult)
            NRM = sb.tile([128, NPG], F32, tag="nrm", bufs=2)
            for o in range(0, NPG, 512):
                w = min(512, NPG - o)
                ssp = pp.tile([128, 512], F32, space="PSUM", name=f"ssp_{o}", tag="ssp", bufs=2)
                nc.tensor.matmul(out=ssp[:, :w], lhsT=SPERM[:], rhs=SQ16[:, o:o + w], start=True, stop=True)
                nc.scalar.activation(NRM[:, o:o + w], ssp[:, :w], ACT.Sqrt)
            nc.vector.tensor_scalar_max(NRM[:], NRM[:], 1e-6)
            nc.vector.reciprocal(NRM[:], NRM[:])
            HN = sb.tile([128, NPG], F32, tag="hn", bufs=3)
            nc.vector.tensor_tensor(out=HN[:], in0=HG[:], in1=NRM[:], op=ALU.mult)
            for g in range(8):
                v = vg[g]
                nc.sync.dma_start(out=contrib.ap()[0:8, g * NPG:g * NPG + v], in_=HN[16 * g:16 * g + 8, 0:v])
                nc.sync.dma_start(out=contrib.ap()[8:16, g * NPG:g * NPG + v], in_=HG[16 * g:16 * g + 8, 0:v])
            mvec = betamask if with_beta else hnmask
            nc.vector.tensor_scalar(out=dsttab[:, 0:NPG], in0=HN[:], scalar1=mvec[:], scalar2=onemask[:],
                                    op0=ALU.mult, op1=ALU.add)
            nc.vector.memset(dsttab[:, NPG:NPG + KMAX], 0.0)
            return HN

        def conv(glob, dsttab, NUMDEN, HGc, HNc, with_beta):
            nc.vector.memset(NUMDEN[:], 0.0)
            tab = sb.tile([128, NPC], F32)
            P = sb.tile([128, RcMax], F32)
            eidxc = sb.tile([128, ENDW // 16], I16, tag="eidxc", bufs=1)
            Gt = sb.tile([128, ENDW], F32)
            bi = -1
            for c in range(NC):
                for g in range(8):
                    nc.sync.dma_start(out=tab[16 * g:16 * g + 16, :], in_=glob.ap()[16 * c:16 * c + 16, :])
                o = Roff[c]
                nc.sync.dma_start(out=eidxc[:], in_=endsidx_d[:, c * (ENDW // 16):(c + 1) * (ENDW // 16)])
                prev_dst = None
                prev_w = 0
                for a in range(0, Rc[c], B):
                    bi += 1
                    _c2, _a2, W, j0, _K = blocks[bi]
                    assert _c2 == c and _a2 == a and W == min(B, Rc[c] - a)
                    SRCt = sb.tile([128, B], F32, name=f"src_{c}_{a}", tag="SRCt", bufs=2)
                    PRODt = sb.tile([128, B], F16, name=f"prd_{c}_{a}", tag="PRODt", bufs=2)
                    DST16t = sb.tile([128, B], F16, name=f"d16_{c}_{a}", tag="DST16t", bufs=2)
                    Mt = sb.tile([128, B], F16, name=f"mt_{c}_{a}", tag="Mt", bufs=2)
                    SCT = sb.tile([128, B], F16, name=f"sct_{c}_{a}", tag="SCT", bufs=2)
                    sidxc = sb.tile([128, B // 16], I16, name=f"sx_{c}_{a}", tag="sidxc", bufs=2)
                    scidc = sb.tile([128, KMAX], I16, name=f"sc_{c}_{a}", tag="scidc", bufs=2)
                    sdat = sb.tile([128, KMAX], F16, name=f"sd_{c}_{a}", tag="sdat", bufs=1)
                    nc.sync.dma_start(out=sidxc[:, :W // 16], in_=srcidx_d[:, (o + a) // 16:(o + a + W) // 16])
                    nc.sync.dma_start(out=scidc[:], in_=scatidx_d[:, bi * KMAX:(bi + 1) * KMAX])
                    nc.sync.dma_start(out=Mt[:, :W], in_=dmask_d[:, o + a:o + a + W])
                    nc.vector.tensor_copy(out=sdat[:], in_=dsttab[:, j0:j0 + KMAX])
                    nc.gpsimd.ap_gather(SRCt[:, :W], tab[:], sidxc[:, :W // 16],
                                        channels=128, num_elems=NPC, d=1, num_idxs=W)
                    nc.gpsimd.local_scatter(SCT[:, :W], sdat[:], scidc[:],
                                            channels=128, num_elems=W, num_idxs=KMAX)
                    init = 0.0 if a == 0 else prev_dst[:, prev_w - 1:prev_w]
                    nc.vector.tensor_tensor_scan(DST16t[:, :W], Mt[:, :W], SCT[:, :W], init, ALU.mult, ALU.add)
                    prev_dst = DST16t
                    prev_w = W
                    nc.vector.tensor_tensor(out=PRODt[:, :W], in0=SRCt[:, :W], in1=DST16t[:, :W], op=ALU.mult)
                    for po in range(0, W, 512):
                        pw = min(512, W - po)
                        alph = pp.tile([128, 512], F32, space="PSUM", name=f"al_{c}_{a}_{po}", tag="alph", bufs=3)
                        nc.tensor.matmul(out=alph[:, :pw], lhsT=LA[:], rhs=PRODt[:, po:po + pw], start=True, stop=True)
                        payp = sb.tile([128, 512], F32, name=f"pp_{c}_{a}_{po}", tag="payp", bufs=3)
                        nc.scalar.activation(payp[:, :pw], alph[:, :pw], ACT.Exp)
                        hp = pp.tile([128, 512], F32, space="PSUM", name=f"hp_{c}_{a}_{po}", tag="hp", bufs=3)
                        nc.tensor.matmul(out=hp[:, :pw], lhsT=LB1[:], rhs=PRODt[:, po:po + pw], start=True, stop=False)
                        nc.tensor.matmul(out=hp[:, :pw], lhsT=LB2[:], rhs=DST16t[:, po:po + pw], start=False, stop=True)
                        nc.vector.tensor_tensor(out=payp[:, :pw], in0=payp[:, :pw], in1=hp[:, :pw], op=ALU.mult)
                        init = 0.0 if (a + po) == 0 else P[:, a + po - 1:a + po]
                        nc.vector.tensor_tensor_scan(P[:, a + po:a + po + pw], payp[:, :pw],
                                                     zcol[:].to_broadcast([128, pw]), init, ALU.add, ALU.add)
                nc.gpsimd.ap_gather(Gt[:], P[:, :Rc[c]], eidxc[:],
                                    channels=128, num_elems=Rc[c], d=1, num_idxs=ENDW)
                nc.vector.tensor_tensor(out=NUMDEN[:], in0=NUMDEN[:], in1=Gt[:, 1:NPG + 1], op=ALU.add)
                nc.vector.tensor_tensor(out=NUMDEN[:], in0=NUMDEN[:], in1=Gt[:, 0:NPG], op=ALU.subtract)
            # analytic self-loop term: NUMDEN += exp(beta*||hn||^2) * (HG on h-rows | 1 on ones-rows)
            SQH = sb.tile([128, NPG], F16, tag="sq16", bufs=2)
            nc.vector.tensor_tensor(out=SQH[:], in0=HNc[:], in1=HNc[:], op=ALU.mult)
            SELFW = sb.tile([128, NPG], F32, tag="nrm", bufs=2)
            for o2 in range(0, NPG, 512):
                w2 = min(512, NPG - o2)
                qp = pp.tile([128, 512], F32, space="PSUM", name=f"qp_{o2}", tag="ssp", bufs=2)
                nc.tensor.matmul(out=qp[:, :w2], lhsT=SPERM[:], rhs=SQH[:, o2:o2 + w2], start=True, stop=True)
                if with_beta:
                    nc.scalar.activation(SELFW[:, o2:o2 + w2], qp[:, :w2], ACT.Exp, scale=betat[:])
                else:
                    nc.scalar.activation(SELFW[:, o2:o2 + w2], qp[:, :w2], ACT.Exp)
            SELFP = sb.tile([128, NPG], F16, tag="sq16", bufs=2)
            nc.vector.tensor_scalar(out=SELFP[:], in0=HGc[:], scalar1=hnmask[:], scalar2=onemask[:],
                                    op0=ALU.mult, op1=ALU.add)
            nc.vector.tensor_tensor(out=SELFP[:], in0=SELFP[:], in1=SELFW[:], op=ALU.mult)
            nc.vector.tensor_tensor(out=NUMDEN[:], in0=NUMDEN[:], in1=SELFP[:], op=ALU.add)

        def h_from_numden(ND):
            ND16 = sb.tile([128, NPG], F16, tag="sq16", bufs=2)
            nc.vector.tensor_copy(out=ND16[:], in_=ND[:])
            SREP = sb.tile([128, NPG], F32, tag="nrm", bufs=2)
            for o in range(0, NPG, 512):
                w = min(512, NPG - o)
                srp = pp.tile([128, 512], F32, space="PSUM", name=f"srp_{o}", tag="ssp", bufs=2)
                nc.tensor.matmul(out=srp[:, :w], lhsT=PERM[:], rhs=ND16[:, o:o + w], start=True, stop=True)
                nc.vector.tensor_copy(out=SREP[:, o:o + w], in_=srp[:, :w])
            nc.vector.tensor_scalar_max(SREP[:], SREP[:], 1e-12)
            nc.vector.reciprocal(SREP[:], SREP[:])
            nc.vector.tensor_tensor(out=ND[:], in0=ND[:], in1=SREP[:], op=ALU.mult)

        # ---- pipeline
        dsttab = sb.tile([128, NPG + KMAX], F16)
        NUMDEN = sb.tile([128, NPG], F32)
        HN1 = build_table(H1G, contrib1, dsttab, with_beta=False)
        nc.gpsimd.collective_compute("AllGather", ALU.bypass, ins=[contrib1.ap().opt()],
                                     outs=[glob1.ap().opt()], replica_groups=[list(range(NC))])
        for _cr in range(conv_reps):
            conv(glob1, dsttab, NUMDEN, H1G, HN1, with_beta=False)
        h_from_numden(NUMDEN)
        H2G = sb.tile([128, NPG], F32)
        nc.vector.tensor_scalar(out=H2G[:], in0=NUMDEN[:], scalar1=hnmask[:], scalar2=None, op0=ALU.mult)
        HN2 = build_table(H2G, contrib2, dsttab, with_beta=True)
        nc.gpsimd.collective_compute("AllGather", ALU.bypass, ins=[contrib2.ap().opt()],
                                     outs=[glob2.ap().opt()], replica_groups=[list(range(NC))])
        conv(glob2, dsttab, NUMDEN, H2G, HN2, with_beta=True)
        h_from_numden(NUMDEN)

        # ---- FC2 + relu (block-diagonal W2)
        H2n16 = sb.tile([128, NPG], F16, tag="sq16", bufs=2)
        nc.vector.tensor_copy(out=H2n16[:], in_=NUMDEN[:])
        H3G = sb.tile([128, NPG], F32, tag="hn", bufs=3)
        for o in range(0, NPG, 512):
            w = min(512, NPG - o)
            f2p = pp.tile([128, 512], F32, space="PSUM", name=f"f2p_{o}", tag="ssp", bufs=2)
            nc.tensor.matmul(out=f2p[:, :w], lhsT=W2t[:], rhs=H2n16[:, o:o + w], start=True, stop=True)
            nc.scalar.activation(H3G[:, o:o + w], f2p[:, :w], ACT.Relu, bias=b2t[:])

        # ---- pooling: scans + boundary gathers
        SUMB = sb.tile([128, NPG + 16], F32)
        MAXB = sb.tile([128, NPG + 16], F32)
        nc.vector.memset(SUMB[:], 0.0)
        nc.vector.memset(MAXB[:], 0.0)
        nc.vector.tensor_tensor_scan(SUMB[:, 4:4 + NPG], H3G[:], zcol[:].to_broadcast([128, NPG]), 0.0, ALU.add, ALU.add)
        nc.vector.tensor_tensor_scan(MAXB[:, 4:4 + NPG], gmask[:], H3G[:], 0.0, ALU.mult, ALU.max)
        sidx = sb.tile([128, SUMW // 16], I16); nc.sync.dma_start(out=sidx[:], in_=sumidx_d[:, :])
        midx = sb.tile([128, MAXW // 16], I16); nc.sync.dma_start(out=midx[:], in_=maxidx_d[:, :])
        GS = sb.tile([128, SUMW], F32)
        GM = sb.tile([128, MAXW], F32)
        nc.gpsimd.ap_gather(GS[:], SUMB[:], sidx[:], channels=128, num_elems=NPG + 16, d=1, num_idxs=SUMW)
        nc.gpsimd.ap_gather(GM[:], MAXB[:], midx[:], channels=128, num_elems=NPG + 16, d=1, num_idxs=MAXW)
        SUMP = sb.tile([128, G], F32)
        nc.vector.tensor_tensor(out=SUMP[:], in0=GS[:, 1:G + 1], in1=GS[:, 0:G], op=ALU.subtract)
        # restack 8 groups -> [16, G, 8], reduce
        PMs = sb.tile([16, G, 8], F32)
        PSs = sb.tile([16, G, 8], F32)
        for g in range(8):
            nc.sync.dma_start(out=PMs[:, :, g:g + 1], in_=GM[16 * g:16 * g + 16, 0:G, None])
            nc.sync.dma_start(out=PSs[:, :, g:g + 1], in_=SUMP[16 * g:16 * g + 16, :, None])
        PM = sb.tile([16, G], F32)
        PS = sb.tile([16, G], F32)
        nc.vector.tensor_reduce(out=PM[:], in_=PMs[:], axis=mybir.AxisListType.X, op=ALU.max)
        nc.vector.tensor_reduce(out=PS[:], in_=PSs[:], axis=mybir.AxisListType.X, op=ALU.add)
        nc.sync.dma_start(out=cpool.ap()[0:16, :], in_=PM[:])
        nc.sync.dma_start(out=cpool.ap()[16:32, :], in_=PS[:])
        nc.gpsimd.collective_compute("AllGather", ALU.bypass, ins=[cpool.ap().opt()],
                                     outs=[gpool.ap().opt()], replica_groups=[list(range(NC))])
        GPLM = sb.tile([16, G, NC], F32)
        GPLS = sb.tile([16, G, NC], F32)
        gp4 = gpool.ap().rearrange("(r two p) f -> two p f r", two=2, p=16)
        nc.sync.dma_start(out=GPLM[:], in_=gp4[0])
        nc.sync.dma_start(out=GPLS[:], in_=gp4[1])
        GMPt = sb.tile([16, G], F32)
        GAPt = sb.tile([16, G], F32)
        nc.vector.tensor_reduce(out=GMPt[:], in_=GPLM[:], axis=mybir.AxisListType.X, op=ALU.max)
        nc.vector.tensor_reduce(out=GAPt[:], in_=GPLS[:], axis=mybir.AxisListType.X, op=ALU.add)
        GPOOL2 = sb.tile([32, G], F32)
        nc.sync.dma_start(out=GPOOL2[0:16, :], in_=GMPt[:])
        nc.sync.dma_start(out=GPOOL2[16:32, :], in_=GAPt[:])
        RCt = sb.tile([32, G], F32); nc.sync.dma_start(out=RCt[:], in_=RC_d[:, :])
        nc.vector.tensor_tensor(out=GPOOL2[:], in0=GPOOL2[:], in1=RCt[:], op=ALU.mult)
        PF16 = sb.tile([32, G], F16)
        nc.vector.tensor_copy(out=PF16[:], in_=GPOOL2[:])
        f3p = pp.tile([2, G], F32, space="PSUM", name="f3p", tag="alph", bufs=3)
        nc.tensor.matmul(out=f3p[:], lhsT=W3t[:], rhs=PF16[:], start=True, stop=True)
        OUTT = sb.tile([2, G], F32)
        nc.vector.tensor_scalar(out=OUTT[:], in0=f3p[:], scalar1=b3t[:], scalar2=None, op0=ALU.add)
        nc.sync.dma_start(out=out_ext[:, :], in_=OUTT[:])

    nc.compile()
    return nc


class SpmdRunner:

    def __init__(self, nc, n_cores=8):
        install_neuronx_cc_hook()
        self.nc = nc
        self.n_cores = n_cores
        assert nc.dbg_addr is None or not nc.dbg_callbacks
        partition_name = nc.partition_id_tensor.name if nc.partition_id_tensor else None
        in_names, out_names, out_avals, zero_outs = [], [], [], []
        for alloc in nc.m.functions[0].allocations:
            if not isinstance(alloc, mybir.MemoryLocationSet):
                continue
            name = alloc.memorylocations[0].name
            if alloc.kind == "ExternalInput":
                if name != partition_name:
                    in_names.append(name)
            elif alloc.kind == "ExternalOutput":
                out_names.append(name)
                shape = tuple(alloc.tensor_shape)
                dtype = mybir.dt.np(alloc.dtype)
                out_avals.append(jax.core.ShapedArray(shape, dtype))
                zero_outs.append(np.zeros(shape, dtype))
        self.dbg_name = nc.dbg_addr.name if nc.dbg_addr is not None else None
        if self.dbg_name is not None and self.dbg_name in in_names:
            pass  # keep; we must supply zeros
        self.in_names = list(in_names)
        self.out_names = out_names
        self.out_avals = out_avals
        self.zero_outs = zero_outs
        n_params = len(in_names)
        n_outs = len(out_avals)
        all_in_names = list(in_names) + list(out_names)
        if partition_name is not None:
            all_in_names.append(partition_name)
        self.partition_name = partition_name

        def _body(*args):
            operands = list(args)
            if partition_name is not None:
                operands.append(partition_id_tensor())
            outs = _bass_exec_p.bind(
                *operands,
                out_avals=tuple(out_avals),
                in_names=tuple(all_in_names),
                out_names=tuple(out_names),
                lowering_input_output_aliases=(),
                sim_require_finite=True,
                sim_require_nnan=True,
                nc=nc,
            )
            return tuple(outs)

        devices = jax.devices()[: n_cores]
        self.mesh = Mesh(np.asarray(devices), ("core",))
        in_specs = (PartitionSpec("core"),) * (n_params + n_outs)
        out_specs = (PartitionSpec("core"),) * n_outs
        donate = tuple(range(n_params, n_params + n_outs))
        self.fn = jax.jit(
            shard_map(_body, mesh=self.mesh, in_specs=in_specs, out_specs=out_specs, check_rep=False),
            donate_argnums=donate, keep_unused=True,
        )

    def prep_inputs(self, in_maps):
        """in_maps: list of dicts per core. Returns device-put concatenated inputs."""
        n = self.n_cores
        if self.dbg_name is not None:
            in_maps = [{**m, self.dbg_name: np.zeros((1, 2), np.uint32)} for m in in_maps]
        concat = [np.concatenate([np.asarray(in_maps[c][nm]) for c in range(n)], axis=0) for nm in self.in_names]
        sh = jax.sharding.NamedSharding(self.mesh, PartitionSpec("core"))
        return [jax.device_put(a, sh) for a in concat]

    def make_zeros(self):
        sh = jax.sharding.NamedSharding(self.mesh, PartitionSpec("core"))
        return [jax.device_put(np.zeros((self.n_cores * z.shape[0], *z.shape[1:]), z.dtype), sh) for z in self.zero_outs]

    def run(self, dev_inputs):
        outs = self.fn(*dev_inputs, *self.make_zeros())
        jax.block_until_ready(outs)
        return outs

    def results(self, outs):
        res = []
        for c in range(self.n_cores):
            d = {}
            for i, name in enumerate(self.out_names):
                d[name] = np.asarray(outs[i]).reshape(self.n_cores, *self.out_avals[i].shape)[c]
            res.append(d)
        return res

    def bench(self, dev_inputs, iters=10, warmup=2):
        for _ in range(warmup):
            self.run(dev_inputs)
        ts = []
        for _ in range(iters):
            t0 = time.perf_counter()
            self.run(dev_inputs)
            ts.append(time.perf_counter() - t0)
        return ts

    def chain(self, dev_inputs, iters, outs=None):
        """Run `iters` chained executions: iteration k's donated output
        buffers are iteration k-1's outputs, so the executions queue
        back-to-back on the NeuronCores with one client sync at the end.
        Returns wall time and the final outputs."""
        if outs is None:
            outs = self.make_zeros()
        t0 = time.perf_counter()
        for _ in range(iters):
            outs = list(self.fn(*dev_inputs, *outs))
        jax.block_until_ready(outs)
        return time.perf_counter() - t0, outs

    def bench_chained(self, dev_inputs, k1=10, k2=60, reps=3):
        """Per-execution device time via the two-point slope
        (T(k2)-T(k1))/(k2-k1), which cancels the constant client<->device
        round-trip latency that a single blocking run would measure.
        Returns the min slope over `reps` repetitions, in seconds."""
        _, outs = self.chain(dev_inputs, 2)  # warmup
        best = float("inf")
        for _ in range(reps):
            t1, outs = self.chain(dev_inputs, k1, outs)
            t2, outs = self.chain(dev_inputs, k2, outs)
            best = min(best, (t2 - t1) / (k2 - k1))
        return best


_CACHE = {}


def kernel(**inputs):
    x = np.asarray(inputs["x"], np.float32)
    edge_index = np.asarray(inputs["edge_index"])
    batch = np.asarray(inputs["batch"])
    cfg = Cfg(N=x.shape[0], E=edge_index.shape[1], G=64, NC=8)
    im, meta = preprocess(cfg, x, edge_index, batch, inputs["W1"], inputs["b1"],
                          inputs["beta2"], inputs["W2"], inputs["b2"], inputs["W3"], inputs["b3"])
    key = (cfg.N, cfg.E, tuple(meta["Rc"]), meta["SCATW"])
    if key not in _CACHE:
        nc = build_kernel(cfg, meta)
        _CACHE[key] = SpmdRunner(nc, n_cores=cfg.NC)
    r = _CACHE[key]
    din = r.prep_inputs(im)
    t0 = time.perf_counter()
    outs = r.run(din)
    wall = time.perf_counter() - t0
    res = r.results(outs)
    kernel.last_wall_s = wall
    kernel.runner = r
    kernel.dev_inputs = din
    out = res[0]["out"].T.astype(np.float32)  # [G, 2]
    return np.ascontiguousarray(out)

